# revision 37
# baseline (speedup 1.0000x reference)
"""Trainium2 Bass kernel for nn_BivariateNormalAttention.

Self-contained: takes FULL inputs (B=16), shards batch across 8 NeuronCores
(2 images/core), runs a Bass/Tile kernel per core, gathers [16,8,56,56].

Pipeline per image:
  conv3x3(512->256)+BN+ReLU -> conv3x3(256->256)+BN+ReLU -> avgpool16 (7x7)
  -> conv3x3(256->128)+BN+ReLU -> conv3x3(128->128)+BN+ReLU -> avgpool3s2
  -> conv3x3(128->64)+BN+ReLU -> fc(576->128) -> bivariate-normal attention.

Convs 1-2 (99.9% of FLOPs) BOTH run as fp8-e4m3 DoubleRowSwInterleave
matmuls (256-deep contraction per instruction, 2x bf16 FLOP rate). x and
the conv1 output (fp8, double-buffered per image) stay resident in SBUF, so
the only bulk HBM traffic is the one-time x load. Weights are scaled x32
before the fp8 cast; the scale folds back in the PSUM->SBUF activation.
fp8/bf16 conv weights use error-feedback (cascade) rounding over each
(cout,cin)'s 9 taps, largest first: the pooled conv error is driven by the
tap-SUM of weight errors, which cascading bounds by the smallest tap's
half-ulp; this cuts final error ~4x vs independent rounding.

The head (convs 3-5 bf16 / fc f32 / attention) processes both images in one
batched pass: conv3..fc batch images along the matmul free dim; the
attention map runs at full 128 partitions (p = rowhalf*64 + img*32 +
out*4 + gmm). Sigmoid is computed as 1/(1+exp(-x)) so every activation fits
one ACT table set (no mid-loop table reloads). The normalized map is
written bf16 for 1-cycle/row selector matmuls. In the timed r_loop the
head is software-pipelined: each iteration runs the PREVIOUS iteration's
head (identical inputs -> identical outputs) so its serial chain overlaps
conv1's PE-heavy phase, with an epilogue head after the loop.
"""
import sys
import numpy as np
import ml_dtypes

for _p in ("/opt/trn_rl_repo", "/root/.axon_site/_ro/trn_rl_repo"):
    if _p not in sys.path:
        sys.path.append(_p)

import concourse.bacc as bacc
import concourse.mybir as mybir
import concourse.tile as tile
from concourse.bass_utils import run_bass_kernel_spmd

F32 = mybir.dt.float32
FP8 = mybir.dt.float8e4
DR = mybir.MatmulPerfMode.DoubleRowSwInterleave

B, C, H, W = 16, 512, 112, 112
OUT, GMM = 8, 4
NCORE = 8
IMG = B // NCORE                 # 2 images per core
HP, WP = H + 2, W + 2            # 114
FLAT = HP * WP                   # 12996
C1LEN = 13008                    # FLAT padded to %16
RS = 4                           # conv strip rows
NSTRIP = H // RS                 # 28
NBLK = 7                         # 4-strip blocks
BROWS = 4 * RS + 2               # 18 rows per x block (incl halo)
XBLEN = 2064                     # BROWS*WP=2052 padded to %16
NFREE = RS * WP                  # 456
H2 = W2 = H // 2                 # 56
SIG2 = float(H) / 2.0            # sigma = 56
LOGR = float(np.log(3.0))
WSCALE = 32.0                    # fp8 weight pre-scale (power of 2)


def build_nc(r_loop=None, worder="ps", psa_bufs=8, variant="full"):
    """Bass program for one core processing IMG images."""
    nc = bacc.Bacc("TRN2", target_bir_lowering=False, debug=False)

    x = nc.dram_tensor("x", [IMG, 4, 128, FLAT], FP8, kind="ExternalInput")
    w1t = nc.dram_tensor("w1t", [128, 9, 2, 2, 256], FP8, kind="ExternalInput")
    w2t = nc.dram_tensor("w2t", [128, 9, 2, 256], FP8, kind="ExternalInput")
    BF16 = mybir.dt.bfloat16
    w3t = nc.dram_tensor("w3t", [128, 9, 2, 128], BF16, kind="ExternalInput")
    w4t = nc.dram_tensor("w4t", [128, 9, 128], BF16, kind="ExternalInput")
    w5t = nc.dram_tensor("w5t", [128, 9, 64], BF16, kind="ExternalInput")
    wfct = nc.dram_tensor("wfct", [64, 9, 128], F32, kind="ExternalInput")
    b1d = nc.dram_tensor("b1d", [128, 2], F32, kind="ExternalInput")
    b2d = nc.dram_tensor("b2d", [128, 2], F32, kind="ExternalInput")
    b3d = nc.dram_tensor("b3d", [128, 1], F32, kind="ExternalInput")
    b4d = nc.dram_tensor("b4d", [128, 1], F32, kind="ExternalInput")
    b5d = nc.dram_tensor("b5d", [64, 1], F32, kind="ExternalInput")
    # head selectors/constants (batched 2-image, 128-partition layout:
    # partition p = q*64 + img*32 + og, q = spatial row half)
    selpd2 = nc.dram_tensor("selpd2", [128, 2, 128], F32, kind="ExternalInput")
    maskd = nc.dram_tensor("maskd", [128, 4], F32, kind="ExternalInput")
    negxd = nc.dram_tensor("negxd", [128, 28], F32, kind="ExternalInput")
    negyd = nc.dram_tensor("negyd", [128, 56], F32, kind="ExternalInput")
    selpaird = nc.dram_tensor("selpaird", [128, 128], F32,
                              kind="ExternalInput")
    selgd2 = nc.dram_tensor("selgd2", [128, 32], BF16,
                            kind="ExternalInput")
    cstd = nc.dram_tensor("cstd", [128, 1], F32, kind="ExternalInput")  # -ln3

    out = nc.dram_tensor("out", [IMG, OUT, H2, W2], F32, kind="ExternalOutput")
    paccd = nc.dram_tensor("paccd", [IMG, 2, 128, 49], F32,
                           kind="ExternalOutput")

    with tile.TileContext(nc) as tc:
        with (
            tc.tile_pool(name="persist", bufs=1) as pp,
            tc.tile_pool(name="et", bufs=4) as etp,
            tc.tile_pool(name="hc", bufs=1) as hc,
            tc.tile_pool(name="att", bufs=1) as attp,
            tc.tile_pool(name="psa", bufs=psa_bufs, space="PSUM") as psa,
        ):
            # ---------------- persistent tiles ----------------
            xb = [pp.tile([128, 4, XBLEN], FP8, name=f"xb{b}", tag=f"xb{b}")
                  for b in range(NBLK)]
            c1 = [pp.tile([128, 2, C1LEN], FP8, name=f"c1_{i}",
                          tag=f"c1_{i}") for i in range(IMG)]
            w1 = pp.tile([128, 9, 2, 2, 256], FP8, tag="w1")
            w2 = pp.tile([128, 9, 2, 256], FP8, tag="w2")
            w3 = pp.tile([128, 9, 2, 128], BF16, tag="w3")
            w4 = pp.tile([128, 9, 128], BF16, tag="w4")
            w5 = pp.tile([128, 9, 64], BF16, tag="w5")
            wfc = pp.tile([64, 9, 128], F32, tag="wfc")
            b1 = pp.tile([128, 2], F32, tag="b1")
            b2 = pp.tile([128, 2], F32, tag="b2")
            b3 = pp.tile([128, 1], F32, tag="b3")
            b4 = pp.tile([128, 1], F32, tag="b4")
            b5 = pp.tile([64, 1], F32, tag="b5")
            selp2 = pp.tile([128, 2, 128], F32, tag="selp2")
            mask4 = pp.tile([128, 4], F32, tag="mask4")
            negx = pp.tile([128, 28], F32, tag="negx")
            negy = pp.tile([128, 56], F32, tag="negy")
            selpair = pp.tile([128, 128], F32, tag="selpair")
            selg2 = pp.tile([128, 32], BF16, tag="selg2")
            cst = pp.tile([128, 1], F32, tag="cst")
            pacc = [[pp.tile([128, 49], F32, name=f"pacc{i}_{c}",
                             tag=f"pacc{i}_{c}")
                     for c in range(2)] for i in range(IMG)]

            # ---------------- prologue (outside r_loop) ----------------
            for tdst, tsrc in ((w1, w1t), (w2, w2t), (w3, w3t), (w4, w4t),
                               (w5, w5t), (wfc, wfct), (b1, b1d), (b2, b2d),
                               (b3, b3d), (b4, b4d), (b5, b5d),
                               (selp2, selpd2), (mask4, maskd),
                               (negx, negxd), (negy, negyd),
                               (selpair, selpaird), (selg2, selgd2),
                               (cst, cstd)):
                nc.gpsimd.dma_start(tdst[:], tsrc[:])
            # zero c1 once: interior rewritten every image, borders stay 0
            for i in range(IMG):
                nc.gpsimd.memset(
                    c1[i][:].rearrange("p c f -> p (c f)").bitcast(F32), 0.0)
            # zero x block pads [2052:2064) once (never written by DMA)
            for b in range(NBLK):
                nc.vector.memset(
                    xb[b][:].rearrange("p c f -> p (c f)").bitcast(F32)
                    .rearrange("p (c f) -> p c f", f=XBLEN // 4)[:, :, 513:516],
                    0.0)

            def load_x(img):
                for b in range(NBLK):
                    nc.gpsimd.dma_start(
                        xb[b][:, :, 0:BROWS * WP],
                        x[img].rearrange("c p f -> p c f")
                        [:, :, 16 * b * WP:16 * b * WP + BROWS * WP])

            def conv1(img):
                for blk in range(NBLK):
                    for co in range(2):
                        ps = [psa.tile([128, NFREE], F32, name="ps",
                                       tag="ps")
                              for _ in range(4)]
                        pairs = [(t, cp) for t in range(9) for cp in range(2)]
                        if worder == "ps":
                            seq = [(p, s) for p in range(18) for s in range(4)]
                        else:
                            seq = [(p, s) for s in range(4) for p in range(18)]
                        for p, s4 in seq:
                            t, cp = pairs[p]
                            base = 4 * s4 * WP + (t // 3) * WP + t % 3
                            nc.tensor.matmul(
                                ps[s4][:],
                                w1[:, t, cp, co, :]
                                .rearrange("p (a b) -> p a b", b=128),
                                xb[blk][:, 2 * cp:2 * cp + 2, base:base + NFREE],
                                start=(p == 0), stop=(p == 17), perf_mode=DR)
                        for s4 in range(4):
                            srow = 4 * (4 * blk + s4)
                            nc.scalar.activation(
                                c1[img][:, co, 0:FLAT]
                                .rearrange("p (r c) -> p r c", c=WP)
                                [:, 1 + srow:5 + srow, 1:113],
                                ps[s4][:].rearrange("p (r c) -> p r c", c=WP)
                                [:, :, 0:112],
                                mybir.ActivationFunctionType.Relu,
                                bias=b1[:, co:co + 1], scale=1.0 / WSCALE)

            def conv2(img):
                for c in range(2):
                    nc.vector.memset(pacc[img][c][:], 0.0)
                for blk in range(NBLK):
                    for co in range(2):
                        ps = [psa.tile([128, NFREE], F32, name="ps",
                                       tag="ps")
                              for _ in range(4)]
                        for t in range(9):
                            for s4 in range(4):
                                s = 4 * blk + s4
                                base = 4 * s * WP + (t // 3) * WP + t % 3
                                nc.tensor.matmul(
                                    ps[s4][:],
                                    w2[:, t, co, :]
                                    .rearrange("p (a b) -> p a b", b=128),
                                    c1[img][:, 0:2, base:base + NFREE],
                                    start=(t == 0), stop=(t == 8),
                                    perf_mode=DR)
                        for s4 in range(4):
                            et = etp.tile([128, RS, WP], mybir.dt.bfloat16,
                                          tag=f"et{co}")
                            nc.scalar.activation(
                                et[:],
                                ps[s4][:].rearrange("p (a b) -> p a b", b=WP),
                                mybir.ActivationFunctionType.Relu,
                                bias=b2[:, co:co + 1], scale=1.0 / WSCALE)
                            rs_ = etp.tile([128, 7], F32, tag=f"rs{co}")
                            nc.vector.reduce_sum(
                                rs_[:],
                                et[:, :, 0:112].rearrange(
                                    "p r (g c) -> p g r c", c=16),
                                axis=mybir.AxisListType.XY)
                            nc.vector.tensor_add(
                                pacc[img][co][:, blk * 7:(blk + 1) * 7],
                                pacc[img][co][:, blk * 7:(blk + 1) * 7],
                                rs_[:])

            def dump_pacc(img):
                for co in range(2):
                    nc.gpsimd.dma_start(paccd[img, co], pacc[img][co][:])

            def head_both():
                ACT = mybir.ActivationFunctionType
                BF = mybir.dt.bfloat16
                # ---- phase A: conv3/4/5 + fc, both images batched in the
                # free dim (img-major blocks of the padded spatial grid) ----
                p3in = []
                for ci in range(2):
                    pi = hc.tile([128, 2, 83], BF, name=f"p3in{ci}",
                                 tag=f"p3in{ci}")
                    nc.vector.memset(pi[:], 0.0)
                    for img in range(IMG):
                        nc.vector.tensor_copy(
                            pi[:, img, 10:73]
                            .rearrange("p (a b) -> p a b", b=9)[:, :, 0:7],
                            pacc[img][ci][:]
                            .rearrange("p (a b) -> p a b", b=7))
                    p3in.append(pi)
                ps3 = psa.tile([128, NFREE], F32, name="ps3",
                               tag="ps")[:, 0:126]
                ps3v = ps3.rearrange("p (i f) -> p i f", f=63)
                k = 0
                for ci in range(2):
                    for t in range(9):
                        off = (t // 3) * 9 + t % 3
                        nc.tensor.matmul(
                            ps3v, w3[:, t, ci, :],
                            p3in[ci][:, :, off:off + 63],
                            start=(k == 0), stop=(k == 17))
                        k += 1
                p4in = hc.tile([128, 2, 83], BF, tag="p4in")
                nc.vector.memset(p4in[:], 0.0)
                for img in range(IMG):
                    nc.scalar.activation(
                        p4in[:, img, 10:73]
                        .rearrange("p (a b) -> p a b", b=9)[:, :, 0:7],
                        ps3v[:, img, :]
                        .rearrange("p (a b) -> p a b", b=9)[:, :, 0:7],
                        ACT.Relu, bias=b3[:, 0:1])
                ps4 = psa.tile([128, NFREE], F32, name="ps4",
                               tag="ps")[:, 0:126]
                ps4v = ps4.rearrange("p (i f) -> p i f", f=63)
                for t in range(9):
                    off = (t // 3) * 9 + t % 3
                    nc.tensor.matmul(ps4v, w4[:, t, :],
                                     p4in[:, :, off:off + 63],
                                     start=(t == 0), stop=(t == 8))
                c4t = hc.tile([128, 2, 49], F32, tag="c4t")
                for img in range(IMG):
                    nc.scalar.activation(
                        c4t[:, img].rearrange("p (a b) -> p a b", b=7),
                        ps4v[:, img, :]
                        .rearrange("p (a b) -> p a b", b=9)[:, :, 0:7],
                        ACT.Relu, bias=b4[:, 0:1])
                # avgpool 3x3 stride 2 (sum; /9 folded into w5)
                a1 = hc.tile([128, 2, 21], F32, tag="a1")
                a2 = hc.tile([128, 2, 9], F32, tag="a2")
                for img in range(IMG):
                    c4v = c4t[:, img].rearrange("p (y x) -> p y x", x=7)
                    a1v = a1[:, img].rearrange("p (y x) -> p y x", x=3)
                    nc.vector.tensor_add(a1v, c4v[:, :, 0:5:2],
                                         c4v[:, :, 1:6:2])
                    nc.vector.tensor_add(a1v, a1v, c4v[:, :, 2:7:2])
                    a2v = a2[:, img].rearrange("p (i j) -> p i j", j=3)
                    nc.vector.tensor_add(a2v, a1v[:, 0:5:2, :],
                                         a1v[:, 1:6:2, :])
                    nc.vector.tensor_add(a2v, a2v, a1v[:, 2:7:2, :])
                p5in = hc.tile([128, 2, 27], BF, tag="p5in")
                nc.vector.memset(p5in[:], 0.0)
                for img in range(IMG):
                    nc.vector.tensor_copy(
                        p5in[:, img, 6:21]
                        .rearrange("p (a b) -> p a b", b=5)[:, :, 0:3],
                        a2[:, img].rearrange("p (a b) -> p a b", b=3))
                ps5 = psa.tile([128, NFREE], F32, name="ps5",
                               tag="ps")[0:64, 0:30]
                ps5v = ps5.rearrange("p (i f) -> p i f", f=15)
                for t in range(9):
                    off = (t // 3) * 5 + t % 3
                    nc.tensor.matmul(ps5v, w5[:, t, :],
                                     p5in[:, :, off:off + 15],
                                     start=(t == 0), stop=(t == 8))
                h5 = hc.tile([64, 2, 9], F32, tag="h5")
                for img in range(IMG):
                    nc.scalar.activation(
                        h5[:, img].rearrange("p (a b) -> p a b", b=3),
                        ps5v[:, img, :]
                        .rearrange("p (a b) -> p a b", b=5)[:, :, 0:3],
                        ACT.Relu, bias=b5[:, 0:1])
                psf = psa.tile([128, NFREE], F32, name="psf",
                               tag="ps")[:, 0:2]
                for t in range(9):
                    nc.tensor.matmul(psf, wfc[:, t, :], h5[:, :, t:t + 1],
                                     start=(t == 0), stop=(t == 8))
                # sigmoid via exp + reciprocal (keeps the ACT engine on
                # the exp_and_others table set: no mid-loop table reloads)
                en = hc.tile([128, 2], F32, tag="en")
                nc.scalar.activation(en[:], psf, ACT.Exp, scale=-1.0)
                ep1 = hc.tile([128, 2], F32, tag="ep1")
                nc.vector.tensor_scalar(ep1[:], en[:], 1.0, None,
                                        mybir.AluOpType.add)
                sig = hc.tile([128, 2], F32, tag="sig")
                nc.vector.reciprocal(sig[:], ep1[:])
                # ---- phase B: params + attention at 128 partitions
                # (p = q*64 + img*32 + og, q = row half) ----
                psl = psa.tile([128, NFREE], F32, name="psl",
                               tag="ps")[:, 0:4]
                for img in range(IMG):
                    sigm = hc.tile([128, 4], F32, tag=f"sigm{img}")
                    nc.vector.tensor_scalar(sigm[:], mask4[:],
                                            sig[:, img:img + 1], None,
                                            mybir.AluOpType.mult)
                    nc.tensor.matmul(psl, selp2[:, img, :], sigm[:],
                                     start=(img == 0), stop=(img == 1))
                hp = hc.tile([128, 4], F32, tag="hp")
                nc.vector.tensor_copy(hp[:], psl)
                r128 = hc.tile([128, 1], F32, tag="r128")
                nc.scalar.activation(r128[:], psl[:, 2:3], ACT.Exp,
                                     bias=cst[:, 0:1])
                rho = hc.tile([128, 1], F32, tag="rho")
                nc.vector.tensor_scalar(rho[:], hp[:, 3:4], -0.8, None,
                                        mybir.AluOpType.add)
                rr = hc.tile([128, 1], F32, tag="rr")
                nc.vector.tensor_mul(rr[:], rho[:], rho[:])
                om = hc.tile([128, 1], F32, tag="om")
                nc.vector.tensor_scalar(om[:], rr[:], -1.0, 1.0,
                                        mybir.AluOpType.mult,
                                        mybir.AluOpType.add)
                iom = hc.tile([128, 1], F32, tag="iom")
                nc.vector.reciprocal(iom[:], om[:])
                den = hc.tile([128, 1], F32, tag="den")
                nc.vector.tensor_scalar(den[:], iom[:],
                                        -0.5 / (SIG2 * SIG2), None,
                                        mybir.AluOpType.mult)
                ai = hc.tile([128, 1], F32, tag="ai")
                nc.vector.tensor_mul(ai[:], den[:], r128[:])
                ir = hc.tile([128, 1], F32, tag="ir")
                nc.vector.reciprocal(ir[:], r128[:])
                bj = hc.tile([128, 1], F32, tag="bj")
                nc.vector.tensor_mul(bj[:], den[:], ir[:])
                cc = hc.tile([128, 1], F32, tag="cc")
                nc.vector.scalar_tensor_tensor(
                    cc[:], den[:], -2.0, rho[:],
                    mybir.AluOpType.mult, mybir.AluOpType.mult)
                dx = hc.tile([128, 28], F32, tag="dx")
                nc.vector.tensor_scalar(dx[:], negx[:], hp[:, 0:1], None,
                                        mybir.AluOpType.add)
                dy = hc.tile([128, 56], F32, tag="dy")
                nc.vector.tensor_scalar(dy[:], negy[:], hp[:, 1:2], None,
                                        mybir.AluOpType.add)
                u = hc.tile([128, 28], F32, tag="u")
                nc.vector.scalar_tensor_tensor(
                    u[:], dx[:], ai[:, 0:1], dx[:],
                    mybir.AluOpType.mult, mybir.AluOpType.mult)
                v = hc.tile([128, 56], F32, tag="v")
                nc.vector.scalar_tensor_tensor(
                    v[:], dy[:], bj[:, 0:1], dy[:],
                    mybir.AluOpType.mult, mybir.AluOpType.mult)
                lt = attp.tile([128, 28, 56], F32, tag="lt")
                nc.vector.scalar_tensor_tensor(
                    lt[:], dx[:].unsqueeze(2).broadcast_to([128, 28, 56]),
                    cc[:, 0:1],
                    dy[:].unsqueeze(1).broadcast_to([128, 28, 56]),
                    mybir.AluOpType.mult, mybir.AluOpType.mult)
                nc.vector.tensor_add(
                    lt[:], lt[:],
                    u[:].unsqueeze(2).broadcast_to([128, 28, 56]))
                nc.vector.tensor_add(
                    lt[:], lt[:],
                    v[:].unsqueeze(1).broadcast_to([128, 28, 56]))
                att = attp.tile([128, 1568], F32, tag="att")
                asum = hc.tile([128, 1], F32, tag="asum")
                nc.scalar.activation(
                    att[:], lt[:].rearrange("p a b -> p (a b)"),
                    ACT.Exp, accum_out=asum[:])
                psq = psa.tile([128, NFREE], F32, name="psq",
                               tag="ps")[:, 0:1]
                nc.tensor.matmul(psq, selpair[:], asum[:],
                                 start=True, stop=True)
                inv = hc.tile([128, 1], F32, tag="inv")
                nc.vector.reciprocal(inv[:], psq)
                # normalize on ACT (scale=inv); bf16 output feeds the
                # 1-cycle/row selector matmuls
                attn = attp.tile([128, 1568], BF, tag="attn")
                nc.scalar.activation(attn[:], att[:],
                                     mybir.ActivationFunctionType.Copy,
                                     scale=inv[:, 0:1])
                # output: per (img, q, chunk-of-7-rows) selector matmul,
                # fp32r moving path; psum->sbuf copies alternate between
                # the ACT and DVE engines, chunk DMAs spread over queues
                obuf = attp.tile([8, 2, 3136], F32, tag="obuf")
                dmaeng = [nc.gpsimd, nc.sync]
                k = 0
                for img in range(IMG):
                    for q in range(2):
                        for ch in range(4):
                            pso = psa.tile([128, NFREE], F32, name="pso",
                                           tag="ps")[0:8, 0:392]
                            nc.tensor.matmul(
                                pso,
                                selg2[:, (img * 2 + q) * 8:
                                      (img * 2 + q) * 8 + 8],
                                attn[:, ch * 392:(ch + 1) * 392],
                                start=True, stop=True)
                            ob = obuf[:, img,
                                      q * 1568 + ch * 392:
                                      q * 1568 + (ch + 1) * 392]
                            if k % 2 == 0:
                                nc.scalar.activation(ob, pso, ACT.Copy)
                            else:
                                nc.vector.tensor_copy(ob, pso)
                            dst = out[img].rearrange("o a b -> o (a b)")[
                                :, q * 1568 + ch * 392:
                                q * 1568 + (ch + 1) * 392]
                            dmaeng[k % 2].dma_start(dst, ob)
                            k += 1

            def emit_convs():
                load_x(0)
                conv1(0)
                load_x(1)
                conv2(0)
                conv1(1)
                conv2(1)

            def emit_body(skew):
                if variant == "full":
                    if skew:
                        # software-pipelined: head of the PREVIOUS
                        # iteration overlaps this iteration's convs (same
                        # inputs each iteration, so outputs are identical;
                        # the epilogue head after the loop finalizes)
                        head_both()
                        emit_convs()
                    else:
                        emit_convs()
                        head_both()
                elif variant == "nohead":
                    load_x(0)
                    conv1(0)
                    load_x(1)
                    conv2(0)
                    dump_pacc(0)
                    conv1(1)
                    conv2(1)
                    dump_pacc(1)
                elif variant == "conv1only":
                    load_x(0)
                    conv1(0)
                    load_x(1)
                    conv1(1)
                elif variant == "conv2only":
                    conv2(0)
                    dump_pacc(0)
                    conv2(1)
                    dump_pacc(1)
                elif variant == "headonly":
                    for i in range(IMG):
                        for c in range(2):
                            nc.vector.memset(pacc[i][c][:], 0.0)
                    head_both()
                else:
                    raise ValueError(variant)

            if r_loop:
                if variant == "full":
                    for i in range(IMG):
                        for c in range(2):
                            nc.vector.memset(pacc[i][c][:], 0.0)
                # 2x-unrolled loop body: halves the For_i all-engine
                # barrier crossings; adjacent bodies overlap via normal
                # point-to-point tile dependencies
                if variant == "full" and r_loop % 2 == 0:
                    with tc.For_i(0, r_loop // 2, 1):
                        emit_body(skew=True)
                        emit_body(skew=True)
                else:
                    with tc.For_i(0, r_loop, 1):
                        emit_body(skew=True)
                if variant == "full":
                    head_both()
            else:
                emit_body(skew=False)
    nc.compile()
    return nc


def prep_inputs(inputs):
    """Host prep: fold BN/pool scales, build device layouts, shard batch."""
    x = inputs["x"]
    eps_s = 1.0 / np.sqrt(np.float32(1.0 + 1e-5))
    FP8NP = ml_dtypes.float8_e4m3

    def fold(w, g):
        s = (g * eps_s).astype(np.float32)
        return (w * s[:, None, None, None]).astype(np.float32)

    def q_cascade(w, qdt):
        # quantization with error-feedback over each (cout,cin)'s 9 taps,
        # largest-|w| first: the pooled (low-frequency) conv error is
        # driven by the tap-SUM of weight errors, which cascading bounds by
        # the last (smallest) tap's half-ulp instead of 9 independent ulps.
        co, ci, _, _ = w.shape
        wf = w.reshape(co, ci, 9)
        order = np.argsort(-np.abs(wf), axis=2)
        ws = np.take_along_axis(wf, order, axis=2)
        out_s = np.zeros_like(ws)
        carry = np.zeros((co, ci), np.float32)
        for t in range(9):
            v = ws[:, :, t] + carry
            q = v.astype(qdt).astype(np.float32)
            carry = v - q
            out_s[:, :, t] = q
        out = np.zeros_like(wf)
        np.put_along_axis(out, order, out_s, axis=2)
        return out.reshape(w.shape)

    def q8_cascade(w):
        return q_cascade(w, ml_dtypes.float8_e4m3)

    w1 = q8_cascade(fold(inputs["w1"], inputs["g1"]) * WSCALE)  # [256,512,3,3]
    w2 = q8_cascade(fold(inputs["w2"], inputs["g2"]) * WSCALE)  # [256,256,3,3]
    BF16NP = ml_dtypes.bfloat16
    w3 = q_cascade(fold(inputs["w3"], inputs["g3"]) / 256.0, BF16NP)
    w4 = q_cascade(fold(inputs["w4"], inputs["g4"]), BF16NP)
    w5 = q_cascade(fold(inputs["w5"], inputs["g5"]) / 9.0, BF16NP)
    wfc = np.asarray(inputs["w_fc"], np.float32)      # [128, 576]
    mw = np.asarray(inputs["mix_w"], np.float32).reshape(OUT, GMM)
    mw = np.exp(mw - mw.max(1, keepdims=True))
    mw = mw / mw.sum(1, keepdims=True)                # softmax over gmm

    # conv weights -> [128(p=cin%128), 9(tap), ncin, cout]
    def wt_layout(w, ncin):
        co = w.shape[0]
        r = w.transpose(1, 2, 3, 0).reshape(ncin, 128, 9, co)  # [ncin,128,9,co]
        return np.ascontiguousarray(r.transpose(1, 2, 0, 3))   # [128,9,ncin,co]

    FP8NP = ml_dtypes.float8_e4m3

    def swi_pack(a, b):
        # a, b: [128, 9, G, 2(coc), 128(m)] fp8 -> [128, 9, G, 2, 256] with
        # raw[..., 2m + i] = (a if i == 0 else b)[..., 127 - m]
        ar = a[..., ::-1]
        br = b[..., ::-1]
        st = np.stack([ar, br], axis=-1)              # [...,128,2]
        return np.ascontiguousarray(st.reshape(*st.shape[:-2], 256))

    w1f = wt_layout(w1, 4)                            # [128,9,4,256] f32
    w1v = w1f.reshape(128, 9, 2, 2, 2, 128)           # [p,t,cp,ab,coc,m]
    w1q = w1v.astype(FP8NP)
    w1t = swi_pack(w1q[:, :, :, 0].transpose(0, 1, 2, 3, 4),
                   w1q[:, :, :, 1].transpose(0, 1, 2, 3, 4))
    # -> [128,9,2(cp),2(coc),256]
    w2f = wt_layout(w2, 2)                             # [128,9,2ci,256]
    w2v = w2f.reshape(128, 9, 2, 2, 128)               # [p,t,ci,coc,m]
    w2q = w2v.astype(FP8NP)
    w2t = swi_pack(w2q[:, :, 0], w2q[:, :, 1])         # [128,9,2coc,256]
    w3t = wt_layout(w3, 2).astype(BF16NP)
    w4t = wt_layout(w4, 1)[:, :, 0, :].astype(BF16NP)
    w5t = wt_layout(w5, 1)[:, :, 0, :].astype(BF16NP)
    wfct = np.ascontiguousarray(
        wfc.reshape(128, 64, 9).transpose(1, 2, 0))

    def bias_chunks(b, nchunk):
        return np.ascontiguousarray(
            np.asarray(b, np.float32).reshape(nchunk, 128).T)

    b1h = bias_chunks(inputs["b1"], 2)
    b2h = bias_chunks(inputs["b2"], 2)
    b3h = np.asarray(inputs["b3"], np.float32).reshape(128, 1)
    b4h = np.asarray(inputs["b4"], np.float32).reshape(128, 1)
    b5h = np.asarray(inputs["b5"], np.float32).reshape(64, 1)

    # head selectors for the 128-partition layout p = q*64 + img*32 + og
    scales = [float(H2 - 1), float(W2 - 1), 2.0 * LOGR, 1.6]
    selp2 = np.zeros((128, 2, 128), np.float32)
    for og in range(32):
        for j in range(4):
            for q in range(2):
                for img in range(IMG):
                    selp2[4 * og + j, img, q * 64 + img * 32 + og] = scales[j]
    mask4 = np.zeros((128, 4), np.float32)
    for r in range(128):
        mask4[r, r % 4] = 1.0
    negx = np.zeros((128, 28), np.float32)
    for p in range(128):
        q = p // 64
        negx[p, :] = -(q * 28 + np.arange(28, dtype=np.float32))
    negy = np.broadcast_to(-np.arange(56, dtype=np.float32),
                           (128, 56)).copy()
    selpair = np.zeros((128, 128), np.float32)
    for r in range(128):
        for c in range(128):
            if r % 64 == c % 64:
                selpair[r, c] = 1.0
    selg2 = np.zeros((128, 32), np.float32)
    for img in range(IMG):
        for q in range(2):
            for o in range(OUT):
                for g in range(GMM):
                    selg2[q * 64 + img * 32 + o * GMM + g,
                          (img * 2 + q) * 8 + o] = mw[o, g]
    selg2 = selg2.astype(BF16NP)
    cst = np.full((128, 1), -LOGR, np.float32)

    xp = np.zeros((B, 4, 128, HP, WP), FP8NP)
    xp[:, :, :, 1:113, 1:113] = np.asarray(x, np.float32).reshape(
        B, 4, 128, H, W).astype(FP8NP)
    xp = xp.reshape(B, 4, 128, FLAT)

    common = {
        "w1t": w1t, "w2t": w2t,
        "w3t": w3t, "w4t": w4t, "w5t": w5t, "wfct": wfct,
        "b1d": b1h, "b2d": b2h, "b3d": b3h, "b4d": b4h, "b5d": b5h,
        "selpd2": selp2, "maskd": mask4, "negxd": negx, "negyd": negy,
        "selpaird": selpair, "selgd2": selg2, "cstd": cst,
    }
    in_maps = []
    for c in range(NCORE):
        m = dict(common)
        m["x"] = np.ascontiguousarray(xp[c * IMG:(c + 1) * IMG])
        in_maps.append(m)
    return in_maps


_CACHE = {}


def kernel(**inputs):
    inputs = {k: np.asarray(v) for k, v in inputs.items()}
    if "nc" not in _CACHE:
        _CACHE["nc"] = build_nc()
    nc = _CACHE["nc"]
    in_maps = prep_inputs(inputs)
    res = run_bass_kernel_spmd(nc, in_maps, core_ids=list(range(NCORE)))
    out = np.concatenate([res.results[c]["out"] for c in range(NCORE)], axis=0)
    return np.ascontiguousarray(out.astype(np.float32))

